# revision 22
# baseline (speedup 1.0000x reference)
"""MiniCPM (MLA-style) attention — Trainium2 Bass kernel, 8-way sharded.

Strategy (tensor-parallel over heads, 5 heads/core; seq-parallel low-rank
phase A with AllGather of the latents):

  Phase A (seq-parallel, 256 rows/core): hs^T arrives pre-transposed from
  the host (bf16); qa/ckv computed in natural layout via bf16 matmuls with
  all weights bulk-DMAed up front; rms_norm; outputs transposed on-chip and
  AllGathered (kv latents f32, q latents + roped k_pe bf16).

  Phase B: K^T (bf16) and V' (f32) built per head from the gathered
  latents.  V' layout per k-tile: [V_h0|..|V_h4|ones64] — the 64 ones
  columns make every PV matmul emit the softmax denominator replicated
  across partitions 64:128 of the attention accumulator, so the epilogue
  is a single DVE divide (no reciprocal / partition-broadcast chain).

  Phase C (per q-block of 512, per head sequentially): scores S^T[k,q]
  computed two k-tiles at a time into a 2-bank PSUM pair, one ACT exp per
  pair (halves ACT instruction overhead), causal tri-mask on DVE, PV
  accumulation with the ones-block stationary.  wo matmuls of the previous
  q-block and Q^T builds of the next q-block are interleaved as PE filler
  so the PE never idles long enough for the HAM clock-gate to re-throttle.

  wo: resident in SBUF; each core computes a full [2048,2560] partial with
  its 320 rows; host sums the 8 partials.
"""

import sys
sys.path.insert(0, "/opt/trn_rl_repo")

from contextlib import ExitStack
from functools import partial

import numpy as np

import concourse.bass as bass
import concourse.bacc as bacc
import concourse.tile as tile
from concourse import mybir
from concourse.bass_utils import run_bass_kernel_spmd
from concourse.masks import make_identity

F32 = mybir.dt.float32
F32R = mybir.dt.float32r
BF16 = mybir.dt.bfloat16
AF = mybir.ActivationFunctionType
ALU = mybir.AluOpType

M = 8                  # cores
S = 2048               # sequence
H = 2560               # hidden
RB = S // M            # 256 rows per core (phase A)
QLR = 768              # q low rank
CKV = 256              # kv low rank (normed part)
QK_ROPE = 32
QK_NOPE = 64
Q_HEAD = 96
V_HEAD = 64
NH = 40
NHL = NH // M          # 5 heads per core
EPS = 1e-6
SM_SCALE = float(Q_HEAD) ** -0.5
NQB = S // 512         # 4 q-blocks
NKT = S // 128         # 16 k-tiles
# V' per k-tile: [ones|V0 | ones|V1 | ... | ones|V4] — every head h reads a
# CONTIGUOUS 128-col stationary window [ones64|V_h] at 128h, so the softmax
# denominator always lands partition-0-aligned in the attention accumulator
# (reciprocal_approx_fast requires a partition-0-based input).
VROW = 640
HC = H // 128              # 20 hidden-dim k-tiles

_cache = {}


def _build():
    nc = bacc.Bacc(trn_type="TRN2", target_bir_lowering=False, debug=False,
                   num_devices=M)

    # ---- I/O ----
    # all bulk inputs host-pre-swizzled to [128, ...] partition-major so the
    # DMAs are fully contiguous (512B-chunk strided reads ran at ~40% of HBM)
    hsT_d = nc.dram_tensor("hsT", [128, HC * RB], BF16, kind="ExternalInput").ap()
    wqa_d = nc.dram_tensor("wqa", [128, HC * QLR], BF16, kind="ExternalInput").ap()
    wkva_d = nc.dram_tensor("wkva", [128, HC * (CKV + QK_ROPE)], BF16,
                            kind="ExternalInput").ap()
    csb_d = nc.dram_tensor("csb_d", [128, 2 * 2 * QK_ROPE], F32,
                           kind="ExternalInput").ap()
    cosT = nc.dram_tensor("cosT", [QK_ROPE, S], F32, kind="ExternalInput").ap()
    ssinT = nc.dram_tensor("ssinT", [QK_ROPE, S], F32, kind="ExternalInput").ap()
    tri = nc.dram_tensor("tri", [128, 128], F32, kind="ExternalInput").ap()
    wqb_l = nc.dram_tensor("wqb_l", [128, 6 * NHL * 128], BF16,
                           kind="ExternalInput").ap()
    wkvk_l = nc.dram_tensor("wkvk_l", [128, 2 * NHL * QK_NOPE], BF16,
                            kind="ExternalInput").ap()
    wkvv_l = nc.dram_tensor("wkvv_l", [128, 2 * NHL * V_HEAD], BF16,
                            kind="ExternalInput").ap()
    wo01_d = nc.dram_tensor("wo01", [128, 2 * H], BF16,
                            kind="ExternalInput").ap()
    wo2_d = nc.dram_tensor("wo2", [QK_NOPE, H], BF16,
                           kind="ExternalInput").ap()
    out_p = nc.dram_tensor("out_p", [S, H], F32, kind="ExternalOutput").ap()

    # single merged AllGather buffer: rows 0:256 kv latents, 256:1024 q
    # latents, 1024:1056 roped k_pe — one mesh instead of two serialized ones
    NAG = CKV + QLR + QK_ROPE   # 1056
    agin_all = nc.dram_tensor("agin_all", [NAG, RB], BF16, kind="Internal").ap()
    agout_all = nc.dram_tensor("agout_all", [M * NAG, RB], BF16,
                               kind="Internal", addr_space="Shared").ap()
    agv = agout_all.rearrange("(r n) c -> n r c", r=M)     # [1056, 8, 256]
    agv_kv = agv[0:CKV]                                    # [256, 8, 256]
    agv_q = agv[CKV:NAG]                                   # [800, 8, 256]

    with ExitStack() as ctx:
        tc = ctx.enter_context(tile.TileContext(nc))

        const = ctx.enter_context(tc.tile_pool(name="const", bufs=1))
        persist = ctx.enter_context(tc.tile_pool(name="persist", bufs=1))
        ps = ctx.enter_context(tc.tile_pool(name="ps", bufs=1, space="PSUM"))
        actx = ExitStack()
        sba = actx.enter_context(tc.tile_pool(name="sba", bufs=1))

        # ---- upfront DMAs: phase-A criticals first ----
        hsT_sb = sba.tile([128, HC, RB], BF16)
        wkva_sb = sba.tile([128, HC, CKV + QK_ROPE], BF16)
        wqa_sb = sba.tile([128, HC, QLR], BF16)
        hsT_v = hsT_d.rearrange("p (t c) -> p t c", t=HC)
        wkva_v = wkva_d.rearrange("p (t c) -> p t c", t=HC)
        wqa_v = wqa_d.rearrange("p (t c) -> p t c", t=HC)
        for g in range(4):
            gs = slice(5 * g, 5 * g + 5)
            nc.sync.dma_start(out=hsT_sb[:, gs], in_=hsT_v[:, gs])
            nc.sync.dma_start(out=wkva_sb[:, gs], in_=wkva_v[:, gs])
        for g in range(4):
            gs = slice(5 * g, 5 * g + 5)
            nc.sync.dma_start(out=wqa_sb[:, gs], in_=wqa_v[:, gs])

        ident = const.tile([128, 128], F32)
        make_identity(nc, ident)
        tri_sb = const.tile([128, 128], BF16)
        nc.gpsimd.dma_start(out=tri_sb, in_=tri)
        eps_t = const.tile([128, 1], F32)
        nc.vector.memset(eps_t, EPS)
        # packed cos/sin (transposed) [64, 2048]: rows 0:32 cosT, 32:64 ssinT
        csT = const.tile([64, S], F32)
        nc.gpsimd.dma_start(out=csT[0:32, :], in_=cosT)
        nc.gpsimd.dma_start(out=csT[32:64, :], in_=ssinT)
        # natural-block cos/ssin [128, 2, 64]
        csb = const.tile([128, 2, 2 * QK_ROPE], F32)
        nc.gpsimd.dma_start(out=csb,
                            in_=csb_d.rearrange("p (t c) -> p t c", t=2))
        wkvk_sb = const.tile([128, 2, NHL * QK_NOPE], BF16)
        nc.gpsimd.dma_start(out=wkvk_sb,
                            in_=wkvk_l.rearrange("p (t c) -> p t c", t=2))
        wkvv_sb = const.tile([128, 2, NHL * V_HEAD], BF16)
        nc.gpsimd.dma_start(out=wkvv_sb,
                            in_=wkvv_l.rearrange("p (t c) -> p t c", t=2))
        # wqb/wo: on the sync ring behind hsT — done before the mesh starts
        wqb_sb = const.tile([128, 6, NHL * 128], BF16)
        nc.sync.dma_start(out=wqb_sb,
                          in_=wqb_l.rearrange("p (t c) -> p t c", t=6))
        w01_sb = const.tile([128, 2, H], BF16)
        nc.sync.dma_start(out=w01_sb,
                          in_=wo01_d.rearrange("p (t c) -> p t c", t=2))
        w2_sb = const.tile([128, H], BF16)
        nc.sync.dma_start(out=w2_sb[0:QK_NOPE, :], in_=wo2_d)
        nc.sync.dma_start(out=w2_sb[QK_NOPE:128, :], in_=wo2_d)

        # ---- persistent K^T / V' ----
        KT = [persist.tile([128, S], BF16, tag=f"KT{h}", name=f"KT{h}")
              for h in range(NHL)]
        Vp = persist.tile([128, NKT, VROW], BF16, tag="Vp")
        nc.vector.memset(Vp, 1.0)

        # ================= PHASE A =================
        # all 120 a-projection matmuls back-to-back; norms/transposes after
        ckv_ps = ps.tile([128, 1024], F32, tag="pr", bufs=2, name="ckv_ps")
        ckv_v = [ckv_ps[:, 0:CKV + QK_ROPE],
                 ckv_ps[:, 512:512 + CKV + QK_ROPE]]
        for hc in range(HC):
            for rt in range(2):
                nc.tensor.matmul(ckv_v[rt],
                                 hsT_sb[:, hc, 128 * rt:128 * rt + 128],
                                 wkva_sb[:, hc, :],
                                 start=(hc == 0), stop=(hc == HC - 1))
        qa0 = ps.tile([128, 1024], F32, tag="pr", bufs=2, name="qa0")
        qa1a = ps.tile([128, 384], F32, tag="a0", name="qa1a")
        qa1b = ps.tile([128, 384], F32, tag="a1", name="qa1b")
        qa_v = [[qa0[:, 0:384], qa0[:, 512:896]], [qa1a, qa1b]]
        for hc in range(HC):
            for rt in range(2):
                for jt in range(2):
                    nc.tensor.matmul(qa_v[rt][jt],
                                     hsT_sb[:, hc, 128 * rt:128 * rt + 128],
                                     wqa_sb[:, hc, 384 * jt:384 * jt + 384],
                                     start=(hc == 0), stop=(hc == HC - 1))

        # ckv rms norm + transpose + staging
        for rt in range(2):
            sq3 = sba.tile([128, CKV], F32, tag="sq", name=f"sq3_{rt}")
            ac = sba.tile([128, 1], F32, tag="st0", name=f"ac_{rt}")
            nc.scalar.activation(sq3, ckv_v[rt][:, 0:CKV], AF.Square, accum_out=ac)
            nc.scalar.activation(ac, ac, AF.Sqrt, bias=eps_t, scale=1.0 / CKV)
            crstd = sba.tile([128, 1], F32, tag="st1", name=f"crstd_{rt}")
            nc.vector.reciprocal_approx_fast(crstd, ac)
            ckvn = sba.tile([128, CKV], F32, tag="ckvn", bufs=2, name=f"ckvn_{rt}")
            nc.vector.tensor_scalar_mul(ckvn, ckv_v[rt][:, 0:CKV], crstd)
            ckvT = sba.tile([128, 2, 128], BF16, tag="ckvT", bufs=2,
                            name=f"ckvT_{rt}")
            for jc in range(2):
                tp = ps.tile([128, 128], F32, tag=("a2" if jc == 0 else "a3"),
                             name=f"tpc_{rt}_{jc}")
                nc.tensor.transpose(tp, ckvn[:, 128 * jc:128 * jc + 128], ident)
                if jc == 0:
                    nc.vector.tensor_copy(ckvT[:, jc, :], tp)
                else:
                    nc.scalar.copy(ckvT[:, jc, :], tp)
            nc.scalar.dma_start(
                out=agin_all[0:CKV].rearrange("(a p) c -> p a c", p=128)
                           [:, :, 128 * rt:128 * rt + 128],
                in_=ckvT)

        # k_pe RoPE (natural layout) then transpose, rows 1024:1056
        for rt in range(2):
            t1 = sba.tile([128, QK_ROPE], F32, tag="kp1", name=f"kp1_{rt}")
            nc.vector.tensor_mul(t1, ckv_v[rt][:, CKV:CKV + QK_ROPE],
                                 csb[:, rt, 0:QK_ROPE])
            t2 = sba.tile([128, QK_ROPE], F32, tag="kp2", name=f"kp2_{rt}")
            nc.vector.tensor_mul(t2[:, 0:16], ckv_v[rt][:, CKV + 16:CKV + 32],
                                 csb[:, rt, QK_ROPE:QK_ROPE + 16])
            nc.vector.tensor_mul(t2[:, 16:32], ckv_v[rt][:, CKV:CKV + 16],
                                 csb[:, rt, QK_ROPE + 16:QK_ROPE + 32])
            kpe = sba.tile([128, QK_ROPE], F32, tag="kp3", name=f"kp3_{rt}")
            nc.vector.tensor_add(kpe, t1, t2)
            tp = ps.tile([128, 128], F32, tag="a2", name=f"tpk_{rt}")
            nc.tensor.transpose(tp[0:QK_ROPE, :], kpe, ident)
            pck = sba.tile([QK_ROPE, 128], BF16, tag="pck", bufs=2,
                           name=f"pck_{rt}")
            nc.vector.tensor_copy(pck, tp[0:QK_ROPE, :])
            nc.scalar.dma_start(out=agin_all[CKV + QLR:NAG,
                                             128 * rt:128 * rt + 128], in_=pck)

        # qa rms norm + transpose + staging (rows 256:1024)
        for rt in range(2):
            sq = sba.tile([128, 384], F32, tag="sq", name=f"sq_{rt}")
            a0s = sba.tile([128, 1], F32, tag="st0", name=f"a0s_{rt}")
            a1s = sba.tile([128, 1], F32, tag="st1", name=f"a1s_{rt}")
            nc.scalar.activation(sq, qa_v[rt][0], AF.Square, accum_out=a0s)
            sq2 = sba.tile([128, 384], F32, tag="sq", name=f"sq2_{rt}")
            nc.scalar.activation(sq2, qa_v[rt][1], AF.Square, accum_out=a1s)
            ssum = sba.tile([128, 1], F32, tag="st2", name=f"ssum_{rt}")
            nc.vector.tensor_add(ssum, a0s, a1s)
            nc.scalar.activation(ssum, ssum, AF.Sqrt, bias=eps_t, scale=1.0 / QLR)
            rstd = sba.tile([128, 1], F32, tag="st3", name=f"rstd_{rt}")
            nc.vector.reciprocal_approx_fast(rstd, ssum)
            qan = sba.tile([128, QLR], F32, tag="qan", bufs=2, name=f"qan_{rt}")
            for jt in range(2):
                nc.vector.tensor_scalar_mul(qan[:, 384 * jt:384 * jt + 384],
                                            qa_v[rt][jt], rstd)
            qanT = sba.tile([128, 6, 128], BF16, tag="qanT", bufs=2,
                            name=f"qanT_{rt}")
            for jc in range(6):
                tp = ps.tile([128, 128], F32, tag=("a2" if jc % 2 == 0 else "a3"),
                             name=f"tpq_{rt}_{jc}")
                nc.tensor.transpose(tp, qan[:, 128 * jc:128 * jc + 128], ident)
                if jc % 2 == 0:
                    nc.vector.tensor_copy(qanT[:, jc, :], tp)
                else:
                    nc.scalar.copy(qanT[:, jc, :], tp)
            nc.scalar.dma_start(
                out=agin_all[CKV:CKV + QLR].rearrange("(a p) c -> p a c", p=128)
                           [:, :, 128 * rt:128 * rt + 128],
                in_=qanT)

        nc.gpsimd.collective_compute(
            "AllGather", mybir.AluOpType.bypass,
            replica_groups=[list(range(M))],
            ins=[agin_all], outs=[agout_all],
        )
        actx.close()
        sbc = ctx.enter_context(tc.tile_pool(name="sbc", bufs=1))

        # ================= PHASE B: K^T and V' =================
        cp_engines = [nc.vector, nc.scalar]
        cpi = [0]

        def rot_copy(dst, src):
            e = cp_engines[cpi[0] % 2]
            cpi[0] += 1
            if e is nc.scalar:
                e.copy(dst, src)
            else:
                e.tensor_copy(dst, src)

        for kb in range(NQB):
            ckt = sbc.tile([128, 2, 2, RB], BF16, tag="ckt", bufs=2,
                           name=f"ckt{kb}")
            for d in range(2):
                nc.sync.dma_start(
                    out=ckt[:, d],
                    in_=agv_kv[128 * d:128 * d + 128,
                               2 * kb:2 * kb + 2, :])
            cks = [ckt[:, c].rearrange("p r c -> p (r c)") for c in range(2)]
            for h in range(NHL):
                kps = ps.tile([128, 512], F32, tag="a2", name=f"kps{kb}_{h}")
                for c in range(2):
                    nc.tensor.matmul(
                        kps[0:QK_NOPE, :],
                        wkvk_sb[:, c, QK_NOPE * h:QK_NOPE * h + QK_NOPE],
                        cks[c],
                        start=(c == 0), stop=(c == 1))
                rot_copy(KT[h][0:QK_NOPE, 512 * kb:512 * kb + 512],
                         kps[0:QK_NOPE, :])
            for t4 in range(4):
                vps = ps.tile([128, 512], F32, tag="a3", name=f"vps{kb}_{t4}")
                for c in range(2):
                    nc.tensor.matmul(
                        vps[:, 0:NHL * V_HEAD],
                        cks[c][:, 128 * t4:128 * t4 + 128],
                        wkvv_sb[:, c, :],
                        start=(c == 0), stop=(c == 1))
                kt = 4 * kb + t4
                rot_copy(
                    Vp[:, kt].rearrange("p (h x) -> p h x", h=NHL)
                    [:, :, 64:128],
                    vps[:, 0:NHL * V_HEAD]
                    .rearrange("p (h x) -> p h x", h=NHL))

        # ================= PHASE C =================
        # roped k_pe rows into K^T (same for all heads)
        for h in range(NHL):
            (nc.sync if h % 2 == 0 else nc.scalar).dma_start(
                out=KT[h][QK_NOPE:Q_HEAD, :].rearrange("p (r c) -> p r c", r=M),
                in_=agv_q[QLR:QLR + QK_ROPE, :, :])

        LATs = {}
        QTs = {}
        aTs = {}
        osb_state = {}

        def make_lat(qb):
            latt = sbc.tile([128, 6, 2, RB], BF16, tag="latC", bufs=3,
                            name=f"latt{qb}")
            for r in range(2):
                (nc.scalar if r == 0 else nc.sync).dma_start(
                    out=latt[:, :, r, :],
                    in_=agv_q[0:QLR].rearrange("(d p) r c -> p d r c", p=128)
                        [:, :, 2 * qb + r, :])
            LATs[qb] = latt
            QTs[qb] = {}

        def qt_chunk(qb, h):
            qs = slice(512 * qb, 512 * qb + 512)
            latt = LATs[qb]
            qps = ps.tile([128, 512], F32, tag="a2", name=f"qps{qb}_{h}")
            for c in range(6):
                nc.tensor.matmul(qps, wqb_sb[:, c, 128 * h:128 * h + 128],
                                 latt[:, c].rearrange("p r c -> p (r c)"),
                                 start=(c == 0), stop=(c == 5),
                                 skip_group_check=True)
            qt = sbc.tile([128, 512], BF16, tag="QT", bufs=15,
                          name=f"qt{qb}_{h}")
            nc.vector.tensor_copy(qt[0:QK_NOPE, :], qps[0:QK_NOPE, :])
            t1 = sbc.tile([QK_ROPE, 512], F32, tag="rp1", bufs=2,
                          name=f"rp1_{qb}_{h}")
            nc.vector.tensor_mul(t1, qps[64:96, :], csT[0:32, qs])
            t2 = sbc.tile([QK_ROPE, 512], F32, tag="rp2", bufs=2,
                          name=f"rp2_{qb}_{h}")
            nc.vector.tensor_mul(t2, qps[96:128, :], csT[32:64, qs])
            nc.gpsimd.tensor_add(qt[QK_NOPE:Q_HEAD, :], t1, t2)
            QTs[qb][h] = qt

        def alloc_aT(qb):
            aT01 = sbc.tile([128, 512], BF16, tag="aT01", bufs=2,
                            name=f"aT01_{qb}")
            aT23 = sbc.tile([128, 512], BF16, tag="aT23", bufs=2,
                            name=f"aT23_{qb}")
            aT4d = sbc.tile([128, 512], BF16, tag="aT4", bufs=2,
                            name=f"aT4d_{qb}")
            aTs[qb] = (aT01, aT23, aT4d)

        def wo_chunk(qb, hc, half):
            aT01, aT23, aT4d = aTs[qb]
            q0 = slice(256 * half, 256 * half + 128)
            q1 = slice(256 * half + 128, 256 * half + 256)
            hcs = slice(512 * hc, 512 * hc + 512)
            opsA = ps.tile([128, 512], F32, tag="a3", name=f"opsA{qb}_{hc}_{half}")
            opsB = ps.tile([128, 512], F32, tag="a1", name=f"opsB{qb}_{hc}_{half}")
            nc.tensor.matmul(opsA, aT01[:, q0], w01_sb[:, 0, hcs],
                             start=True, stop=False, skip_group_check=True)
            nc.tensor.matmul(opsB, aT01[:, q1], w01_sb[:, 0, hcs],
                             start=True, stop=False, skip_group_check=True)
            nc.tensor.matmul(opsA, aT23[:, q0], w01_sb[:, 1, hcs],
                             start=False, stop=False, skip_group_check=True)
            nc.tensor.matmul(opsB, aT23[:, q1], w01_sb[:, 1, hcs],
                             start=False, stop=False, skip_group_check=True)
            nc.tensor.matmul(opsA, aT4d[0:QK_NOPE, q0], w2_sb[0:QK_NOPE, hcs],
                             start=False, stop=True, skip_group_check=True)
            nc.tensor.matmul(opsB, aT4d[QK_NOPE:128, q1], w2_sb[QK_NOPE:128, hcs],
                             start=False, stop=True, skip_group_check=True)
            osb = sbc.tile([128, 2, 512], F32, tag="osb", bufs=3,
                           name=f"osb{qb}_{hc}_{half}")
            nc.vector.tensor_copy(osb[:, 0, :], opsA)
            nc.vector.tensor_copy(osb[:, 1, :], opsB)
            (nc.sync if hc % 2 == 0 else nc.gpsimd).dma_start(
                out=out_p.rearrange("(d p) c -> p d c", p=128)
                         [:, 4 * qb + 2 * half:4 * qb + 2 * half + 2, hcs],
                in_=osb)

        def attn_head(qb, h, fills):
            nkt = 4 * qb + 4
            att = ps.tile([128, 512], F32, tag="a0", name=f"att{qb}_{h}")
            QT = QTs[qb][h]

            def emit_pv(state):
                pt2, kt0, kt1, o0, o1 = state
                for kt, o, base in ((kt0, o0, 0), (kt1, o1, 512)):
                    nc.tensor.matmul(att[:, o:512],
                                     Vp[:, kt, 128 * h:128 * h + 128],
                                     pt2[:, base + o:base + 512],
                                     start=(kt == 0), stop=(kt == nkt - 1),
                                     skip_group_check=True)

            prev = None
            for p in range(nkt // 2):
                kt0, kt1 = 2 * p, 2 * p + 1
                o0 = max(0, 128 * kt0 - 512 * qb)
                o1 = max(0, 128 * kt1 - 512 * qb)
                sps = ps.tile([128, 1024], F32, tag="pr", bufs=2,
                              name=f"sps{qb}_{h}_{p}")
                nc.tensor.matmul(sps[:, o0:512],
                                 KT[h][0:Q_HEAD, 128 * kt0:128 * kt0 + 128],
                                 QT[0:Q_HEAD, o0:512],
                                 start=True, stop=True, skip_group_check=True)
                nc.tensor.matmul(sps[:, 512 + o1:1024],
                                 KT[h][0:Q_HEAD, 128 * kt1:128 * kt1 + 128],
                                 QT[0:Q_HEAD, o1:512],
                                 start=True, stop=True, skip_group_check=True)
                pt2 = sbc.tile([128, 1024], BF16, tag="pt", bufs=4,
                               name=f"pt{qb}_{h}_{p}")
                nc.scalar.activation(pt2[:, o0:1024],
                                     sps[:, o0:1024], AF.Exp, scale=SM_SCALE)
                if 128 * kt0 >= 512 * qb:   # diagonal pair
                    nc.gpsimd.tensor_mul(pt2[:, o0:o0 + 128],
                                         pt2[:, o0:o0 + 128], tri_sb)
                    nc.gpsimd.tensor_mul(
                        pt2[:, 512 + o1:512 + o1 + 128],
                        pt2[:, 512 + o1:512 + o1 + 128], tri_sb)
                if prev is not None:
                    emit_pv(prev)
                if fills:
                    fills.pop(0)()
                prev = (pt2, kt0, kt1, o0, o1)
            emit_pv(prev)
            # softmax division: denom is replicated on partitions 64:128
            aT01, aT23, aT4d = aTs[qb]
            # window is [ones|V]: denominator rows 0:64, attention 64:128
            rd = sbc.tile([QK_NOPE, 512], F32, tag="rd", bufs=2,
                          name=f"rd{qb}_{h}")
            nc.vector.reciprocal_approx_fast(rd, att[0:64, :])
            if h < 2:
                nc.vector.tensor_mul(aT01[64 * h:64 * h + 64, :],
                                     att[64:128, :], rd)
            elif h < 4:
                nc.vector.tensor_mul(aT23[64 * (h - 2):64 * (h - 2) + 64, :],
                                     att[64:128, :], rd)
            else:
                nc.vector.tensor_mul(aT4d[0:QK_NOPE, :], att[64:128, :], rd)
                nc.vector.tensor_mul(aT4d[QK_NOPE:128, :], att[64:128, :], rd)

        make_lat(0)
        for h in range(NHL):
            qt_chunk(0, h)
        for qb in range(NQB):
            alloc_aT(qb)
            fills = []
            if qb == 0:
                make_lat(1)
                make_lat(2)
                fills += [partial(qt_chunk, 1, h) for h in range(NHL)]
                fills += [partial(qt_chunk, 2, h) for h in range(NHL)]
            elif qb == 1:
                make_lat(3)
            elif qb == 2:
                fills += [partial(qt_chunk, 3, h) for h in range(NHL)]
            if qb > 0:
                fills += [partial(wo_chunk, qb - 1, hc, half)
                          for hc in range(5) for half in range(2)]
            for h in range(NHL):
                attn_head(qb, h, fills)
            for f in fills:
                f()
        for hc in range(5):
            for half in range(2):
                wo_chunk(NQB - 1, hc, half)

    nc.compile()
    return nc


def _prep(inputs):
    import ml_dtypes
    hs = np.ascontiguousarray(np.asarray(inputs["hidden_states"], np.float32)[0])
    cos = np.asarray(inputs["cos"], np.float32)
    sin = np.asarray(inputs["sin"], np.float32)
    wq_a = np.asarray(inputs["wq_a"], np.float32)
    q_ln = np.asarray(inputs["q_a_ln_w"], np.float32)
    wq_b = np.asarray(inputs["wq_b"], np.float32)
    wkv_a = np.asarray(inputs["wkv_a"], np.float32)
    kv_ln = np.asarray(inputs["kv_a_ln_w"], np.float32)
    wkv_b = np.asarray(inputs["wkv_b"], np.float32)
    wo = np.asarray(inputs["wo"], np.float32)

    if not np.all(q_ln == 1.0):
        wq_b = wq_b * q_ln[:, None]
    if not np.all(kv_ln == 1.0):
        wkv_b = wkv_b * kv_ln[:, None]

    ssin = np.concatenate([-sin[:, :16], sin[:, 16:]], axis=1)
    cosT = np.ascontiguousarray(cos.T)
    ssinT = np.ascontiguousarray(ssin.T)
    tri = np.triu(np.ones((128, 128), np.float32))

    def sw(a, p=128):
        # [T*p, C] -> [p, T*C]: partition-major swizzle for contiguous DMA
        t = a.shape[0] // p
        return np.ascontiguousarray(
            a.reshape(t, p, -1).transpose(1, 0, 2).reshape(p, -1))

    wqa_bf = sw(np.ascontiguousarray(wq_a).astype(ml_dtypes.bfloat16))
    wkva_bf = sw(np.ascontiguousarray(wkv_a).astype(ml_dtypes.bfloat16))

    in_maps = []
    for c in range(M):
        heads = range(NHL * c, NHL * c + NHL)
        qb_cols = []
        for h in heads:
            qb_cols.extend(range(96 * h, 96 * h + 96))
            # swapped pe columns: [16:32] then [0:16] of the pe block
            qb_cols.extend(range(96 * h + 80, 96 * h + 96))
            qb_cols.extend(range(96 * h + 64, 96 * h + 80))
        wqb_loc = sw(np.ascontiguousarray(wq_b[:, qb_cols])
                     .astype(ml_dtypes.bfloat16))
        kcols, vcols = [], []
        for h in heads:
            kcols.extend(range(128 * h, 128 * h + 64))
            vcols.extend(range(128 * h + 64, 128 * h + 128))
        wo_c = wo[NHL * V_HEAD * c:NHL * V_HEAD * (c + 1)]
        csb_host = np.concatenate(
            [sw(np.ascontiguousarray(cos[RB * c:RB * c + RB])).reshape(128, 2, 32),
             sw(np.ascontiguousarray(ssin[RB * c:RB * c + RB])).reshape(128, 2, 32)],
            axis=2).reshape(128, 128)
        in_maps.append({
            "hsT": sw(np.ascontiguousarray(
                hs[RB * c:RB * c + RB].T).astype(ml_dtypes.bfloat16)),
            "csb_d": np.ascontiguousarray(csb_host),
            "cosT": cosT,
            "ssinT": ssinT,
            "tri": tri,
            "wqa": wqa_bf,
            "wkva": wkva_bf,
            "wqb_l": wqb_loc,
            "wkvk_l": sw(np.ascontiguousarray(wkv_b[:, kcols])
                         .astype(ml_dtypes.bfloat16)),
            "wkvv_l": sw(np.ascontiguousarray(wkv_b[:, vcols])
                         .astype(ml_dtypes.bfloat16)),
            "wo01": sw(np.ascontiguousarray(wo_c[0:256])
                       .astype(ml_dtypes.bfloat16)),
            "wo2": np.ascontiguousarray(wo_c[256:320])
                   .astype(ml_dtypes.bfloat16),
        })
    return in_maps


def kernel(**inputs):
    if "nc" not in _cache:
        _cache["nc"] = _build()
    nc = _cache["nc"]
    in_maps = _prep(inputs)
    res = run_bass_kernel_spmd(nc, in_maps, core_ids=list(range(M)))
    out = res.results[0]["out_p"].astype(np.float32)
    for c in range(1, M):
        out += res.results[c]["out_p"]
    return out.reshape(1, S, H)
